# revision 10
# baseline (speedup 1.0000x reference)
"""Causal self-attention on 8 Trainium2 NeuronCores.

Sharding: core = (batch b in {0,1}) x (head-group g in {0..3}), 4 heads per
core. Each core computes qkv for its heads from x[b], runs causal attention,
and multiplies by its 256 rows of w_proj, producing a partial [T, C] output
in bf16. Host sums the 4 partials per batch in f32.

Layout: everything is computed "transposed" so no on-chip transposes are
needed. The host feeds x[b].T in bf16; q^T/k^T come out of the qkv matmul
with head-dim on partitions (exactly the S^T = K Q^T operand layout); softmax
is done on S^T (keys on partitions, queries on free); the A@V output Y^T is
exactly the lhsT layout the final projection needs.

v4 (vs the 165us v3):
- AV is col-tiled: head 2p -> array cols 0-63, head 2p+1 -> cols 64-127, the
  two streams run concurrently, halving AV wall time AND landing Y^T with
  the pair's two heads stacked [128, 512] -- which kills the ytr/sums
  partition-shift DMAs outright.
- the softmax denominator (previously a 65th ones-column on V, which blocked
  col-tiling) is now: gpsimd accumulates es into an f32 essum per (pair,qc)
  off the critical path; one tiny PE matmul (ones^T @ essum) reduces it, two
  concurrent K=1 matmuls broadcast the reciprocal to 128 partitions.
- the causal mask is a post-exp DVE multiply with a 0/1 triangle instead of
  PE matmuls accumulating -inf blocks pre-exp.
- exp (ACT) runs one kt ahead of the AV that consumes it, so the ~68us of
  total exp time stays off the PE's critical path.
- no heaters: the drip-feed of B/proj/norm work keeps the PE HAM-warm
  through the drain.
"""

import numpy as np
import ml_dtypes

import concourse.bass as bass
import concourse.bacc as bacc
import concourse.tile as tile
from concourse import mybir
from concourse.bass_utils import run_bass_kernel_spmd

F32 = mybir.dt.float32
BF16 = mybir.dt.bfloat16
EXP = mybir.ActivationFunctionType.Exp
COPY = mybir.ActivationFunctionType.Copy
BF16NP = ml_dtypes.bfloat16

B, T, C, H, HD = 2, 2048, 1024, 16, 64
NCORES = 8
HPC = 4      # heads per core
NPAIR = 2    # head pairs per core
NCT = C // 128   # 8 c-tiles
NTT = T // 128   # 16 t-tiles
NQC = T // 512   # 4 query chunks
SCALE = 1.0 / np.sqrt(HD)


def build_kernel():
    nc = bacc.Bacc("TRN2", target_bir_lowering=False, debug=False, num_devices=NCORES)

    xT = nc.dram_tensor("xT", [C, T], BF16, kind="ExternalInput")
    wqk = nc.dram_tensor("wqk", [C, 512], BF16, kind="ExternalInput")
    wv = nc.dram_tensor("wv", [C, 256], BF16, kind="ExternalInput")
    wp = nc.dram_tensor("wp", [256, C], BF16, kind="ExternalInput")
    trimask = nc.dram_tensor("trimask", [128, 256], BF16, kind="ExternalInput")
    ones = nc.dram_tensor("ones", [128, 128], BF16, kind="ExternalInput")
    out = nc.dram_tensor("out", [T, C], BF16, kind="ExternalOutput")

    with tile.TileContext(nc) as tc:
        _body(tc, xT, wqk, wv, wp, trimask, ones, out)

    nc.compile()
    return nc


def _body(tc, xT, wqk, wv, wp, trimask, ones, out):
    nc = tc.nc
    from contextlib import ExitStack

    with ExitStack() as ctx:
        sb = lambda name: ctx.enter_context(tc.tile_pool(name=name, bufs=1))
        qkT_sb = sb("qkT").tile([128, 4 * T], BF16)       # bands q0,k0,q1,k1
        v_sb = sb("v").tile([128, NTT * 256], BF16)       # per k-tile: 4 heads x 64
        yt_sb = sb("yt").tile([128, NPAIR * T], BF16)     # pair p: rows 0-63 head 2p, 64-127 head 2p+1
        wp_sb = sb("wp").tile([128, 2 * C], BF16)
        trimask_sb = sb("trimask").tile([128, 256], BF16)
        ones_sb = sb("ones").tile([128, 128], BF16)

        es_pool = ctx.enter_context(tc.tile_pool(name="es", bufs=3))
        essum_pool = ctx.enter_context(tc.tile_pool(name="essum", bufs=2))
        rc_pool = ctx.enter_context(tc.tile_pool(name="rc", bufs=2))
        ost_pool = ctx.enter_context(tc.tile_pool(name="ost", bufs=3))

        # all-ones f32 column for the fp32 denominator matmuls
        onesf_sb = sb("onesf").tile([128, 2], F32)
        nc.vector.memset(onesf_sb[:], 1.0)

        # PSUM (8 banks): psS 2 slots x [128,1024] = 4, av 2 x [128,512] = 2,
        # misc (B psv / proj pso / d / psR rotate) 2 x [128,512] = 2.
        ps = ctx.enter_context(tc.tile_pool(name="ps", bufs=2, space="PSUM"))
        av_pool = ctx.enter_context(tc.tile_pool(name="av", bufs=2, space="PSUM"))
        psS_pool = ctx.enter_context(tc.tile_pool(name="psS", bufs=2, space="PSUM"))

        xw_pool = ctx.enter_context(tc.tile_pool(name="xw", bufs=1))
        xT_sb = xw_pool.tile([128, NCT * T], BF16, name="xT_sb")
        wqk_sb = xw_pool.tile([128, NCT * 512], BF16, name="wqk_sb")
        wv_sb = xw_pool.tile([128, NCT * 256], BF16, name="wv_sb")

        # input DMAs: few big instructions (each costs ~600ns issue time)
        def load_w(w, w_sb, cols, c0, ncx):
            dst = w_sb[:, c0 * cols:(c0 + ncx) * cols].rearrange("p (c w) -> p c w", c=ncx)
            src = w[c0 * 128:(c0 + ncx) * 128, :].rearrange("(c p) w -> p c w", c=ncx, p=128)
            nc.gpsimd.dma_start(dst, src)

        # x even c-tiles on sync, odd on scalar (idle until the first exp);
        # weights on gpsimd -- three DGE pipelines ramp in parallel so stage
        # A's c-loop never starves. c0 is split in halves so the first
        # stage-A matmuls can start as early as possible.
        def load_x(c0, eng, tlo=0, thi=T):
            eng.dma_start(xT_sb[:, c0 * T + tlo:c0 * T + thi],
                          xT[c0 * 128:(c0 + 1) * 128, tlo:thi])

        load_w(wqk, wqk_sb, 512, 0, 1)
        load_x(0, nc.sync, 0, 1024)
        load_x(0, nc.scalar, 1024, 2048)
        load_w(wv, wv_sb, 256, 0, 1)
        load_x(1, nc.sync)
        load_w(wqk, wqk_sb, 512, 1, 3)
        load_w(wv, wv_sb, 256, 1, 3)
        load_x(2, nc.scalar)
        load_x(3, nc.sync)
        load_w(wqk, wqk_sb, 512, 4, 4)
        load_w(wv, wv_sb, 256, 4, 4)
        load_x(4, nc.scalar)
        load_x(5, nc.sync)
        load_x(6, nc.scalar)
        load_x(7, nc.sync)
        nc.gpsimd.dma_start(
            wp_sb[:].rearrange("p (c w) -> p c w", c=2),
            wp.ap().rearrange("(c p) w -> p c w", c=2, p=128))
        nc.gpsimd.dma_start(trimask_sb[:], trimask[:])
        nc.gpsimd.dma_start(ones_sb[:], ones[:])

        # ---- stage A: qkT bands [128, T] = w_band^T @ xT ----
        # c-outer over two-band passes: all 8 PSUM banks hold the 8
        # accumulating t-chunks, each c-tile of xT feeds 8 matmuls the
        # moment its DMA lands, and each lhsT load feeds 4 matmuls.
        for pass_ in range(2):
            bands = (2 * pass_, 2 * pass_ + 1)
            accs = {}
            for b in bands:
                acc01 = psS_pool.tile([128, 1024], F32, tag="psS", name=f"accA_{b}_01")
                acc2 = av_pool.tile([128, 512], F32, tag="av", name=f"accA_{b}_2")
                acc3 = ps.tile([128, 512], F32, tag="ps", name=f"accA_{b}_3")
                accs[b] = [acc01, acc2, acc3]
            for c in range(NCT):
                for b in bands:
                    lhs = wqk_sb[:, c * 512 + b * 128: c * 512 + (b + 1) * 128]
                    acc01, acc2, acc3 = accs[b]
                    dsts = [acc01[:, 0:512], acc01[:, 512:1024], acc2[:], acc3[:]]
                    for t4 in range(4):
                        nc.tensor.matmul(
                            dsts[t4], lhs,
                            xT_sb[:, c * T + t4 * 512: c * T + (t4 + 1) * 512],
                            start=(c == 0), stop=(c == NCT - 1))
            for b in bands:
                acc01, acc2, acc3 = accs[b]
                nc.vector.tensor_copy(qkT_sb[:, b * T: b * T + 1024], acc01[:])
                nc.vector.tensor_copy(qkT_sb[:, b * T + 1024: b * T + 1536], acc2[:])
                nc.vector.tensor_copy(qkT_sb[:, b * T + 1536: b * T + 2048], acc3[:])

        # ---- stage B: v natural [t, j]; tiles 0-3 now, the rest
        # drip-fed into stage C as PE filler (keeps HAM at full clock) --
        B_psv = {}

        def emit_B_half(t, half):
            # halved so one drip-pop (~430ns of PE) fits the per-kt slack
            # under the exp cadence without stalling the S pipeline
            if half == 0:
                B_psv[t] = ps.tile([128, 512], F32, tag="ps", name=f"psv_{t}")
            psv = B_psv[t]
            for c in range(4 * half, 4 * half + 4):
                lhs = xT_sb[:, c * T + t * 128: c * T + (t + 1) * 128]
                nc.tensor.matmul(psv[:, 0:256], lhs, wv_sb[:, c * 256:(c + 1) * 256],
                                 start=(c == 0), stop=(c == NCT - 1))
            if half == 1:
                nc.vector.tensor_copy(v_sb[:, t * 256:(t + 1) * 256], psv[:, 0:256])
                del B_psv[t]

        def emit_B(t):
            emit_B_half(t, 0)
            emit_B_half(t, 1)

        for t in range(4):
            emit_B(t)

        # ---- stage C: attention; stage D: projection. All cross-chunk
        # serial work (denominator chain, projection) is drip-fed into the
        # NEXT chunk's kt loop so the in-order PE stream never stalls >3.4us
        # (a fully-idle HAM window would halve the PE clock for the rest of
        # the attention phase).
        def emit_proj_group(t, tail=False):
            ost = ost_pool.tile([128, 1024], BF16, tag="ost", name=f"ost_{t}")
            for n in range(2):
                pso = ps.tile([128, 512], F32, tag="ps", name=f"pso_{t}_{n}")
                for p in range(NPAIR):
                    lhsT = yt_sb[:, p * T + t * 128: p * T + (t + 1) * 128]
                    rhs = wp_sb[:, p * C + n * 512: p * C + (n + 1) * 512]
                    nc.tensor.matmul(pso[:], lhsT, rhs, start=(p == 0), stop=(p == NPAIR - 1))
                if tail and n == 0:
                    # post-attention drain: ACT is idle, split the PSUM
                    # evictions across scalar+vector
                    nc.scalar.activation(ost[:, 0:512], pso[:], COPY, scale=1.0)
                else:
                    nc.vector.tensor_copy(ost[:, n * 512:(n + 1) * 512], pso[:])
            nc.sync.dma_start(out[t * 128:(t + 1) * 128, :], ost[:])

        # Drip-feed scheduling on a global step counter g (one step per kt
        # iteration across the whole attention phase). Each pending item
        # carries a threshold step; the pop scans for the first eligible
        # item so late-deadline items don't head-of-line-block early ones.
        # B(t) is deadlined to just before the AV that reads v[t].
        OFF_P0 = {0: 0, 1: 8, 2: 24, 3: 48}
        pending = [(max(0, OFF_P0[t // 4] + t - 8 + half), lambda t=t, half=half: emit_B_half(t, half))
                   for t in range(4, NTT) for half in range(2)]
        g = 0

        for qc in (0, 1, 2, 3):
            nkt = 4 * qc + 4
            for p in range(NPAIR):
                qb, kb = 2 * p, 2 * p + 1
                # col-tiled AV: each head's accumulation group must own its
                # bank (two concurrent groups can't share a zero region),
                # h0 lands on partitions 0-63 of bank a, h1 on 64-127 of b.
                av_a = av_pool.tile([128, 512], F32, tag="av", name=f"ava_{p}_{qc}")
                av_b = av_pool.tile([128, 512], F32, tag="av", name=f"avb_{p}_{qc}")
                essum = essum_pool.tile([128, 1024], F32, tag="essum", name=f"essum_{p}_{qc}")

                def emit_S(kt, p=p, qb=qb, kb=kb, qc=qc):
                    psb = psS_pool.tile([128, 1024], F32, tag="psS", name=f"psS_{p}_{qc}_{kt}")
                    d = kt - 4 * qc
                    slo = max(d, 0) * 128
                    for h in range(2):
                        base = 64 * h
                        lhsT = qkT_sb[base:base + 64, kb * T + kt * 128: kb * T + (kt + 1) * 128]
                        rhs = qkT_sb[base:base + 64, qb * T + qc * 512 + slo: qb * T + (qc + 1) * 512]
                        nc.tensor.matmul(psb[:, h * 512 + slo:(h + 1) * 512], lhsT, rhs,
                                         start=True, stop=True, tile_position=(base, 0))
                    return psb

                def emit_exp(kt, psb, p=p, qc=qc, essum=essum):
                    # exp of S^T (ACT), then 0/1-triangle mask on the
                    # diagonal block (DVE), then accumulate into the f32
                    # denominator tile (gpsimd, far off the critical path).
                    d = kt - 4 * qc
                    lo = max(d, 0) * 128
                    es = es_pool.tile([128, 1024], BF16, tag="es", name=f"es_{p}_{qc}_{kt}")
                    es2 = es[:].rearrange("p (h q) -> p h q", h=2, q=512)
                    psb2 = psb[:].rearrange("p (h q) -> p h q", h=2, q=512)
                    nc.scalar.activation(es2[:, :, lo:], psb2[:, :, lo:], EXP, scale=SCALE)
                    if d >= 0:
                        tm = trimask_sb[:].rearrange("p (h q) -> p h q", h=2, q=128)
                        nc.vector.tensor_mul(es2[:, :, lo:lo + 128], es2[:, :, lo:lo + 128], tm)
                    esum2 = essum[:].rearrange("p (h q) -> p h q", h=2, q=512)
                    if kt == 0:
                        nc.gpsimd.tensor_copy(esum2[:, :, lo:], es2[:, :, lo:])
                    else:
                        nc.gpsimd.tensor_tensor(esum2[:, :, lo:], esum2[:, :, lo:], es2[:, :, lo:],
                                                mybir.AluOpType.add)
                    return es

                def emit_AV(kt, es, p=p, qc=qc, av_a=av_a, av_b=av_b, nkt=nkt):
                    d = kt - 4 * qc
                    lo = max(d, 0) * 128
                    for h, av in ((0, av_a), (1, av_b)):
                        hh = 2 * p + h
                        lhsT_v = v_sb[:, kt * 256 + hh * 64: kt * 256 + (hh + 1) * 64]
                        nc.tensor.matmul(av[64 * h:64 * (h + 1), lo:], lhsT_v,
                                         es[:, h * 512 + lo:(h + 1) * 512],
                                         start=(kt == 0), stop=(kt == nkt - 1),
                                         tile_position=(0, 64 * h))

                # software pipeline: S runs 2 kts ahead, exp 1 kt ahead of
                # the AV that consumes it, so ACT (the exp-rate floor of the
                # whole attention phase) never blocks the PE.
                pipe = [emit_S(0)]
                if nkt > 1:
                    pipe.append(emit_S(1))
                es_cur = emit_exp(0, pipe[0])
                for kt in range(nkt):
                    pipe.pop(0)
                    if kt + 2 < nkt:
                        pipe.append(emit_S(kt + 2))
                    es_next = emit_exp(kt + 1, pipe[0]) if kt + 1 < nkt else None
                    emit_AV(kt, es_cur)
                    es_cur = es_next
                    # pop at most ONE filler per kt: a 2-item burst delays
                    # S(kt+2) past the exp cadence and stalls the pipeline
                    i = 0
                    while i < len(pending):
                        if pending[i][0] <= g:
                            pending.pop(i)[1]()
                            break
                        i += 1
                    g += 1

                # gather Y^T to SBUF inline so the pair's av banks free
                # before the next pair's first AV (the av ring is 2 deep)
                avs = rc_pool.tile([128, 512], F32, tag="avs", name=f"avs_{p}_{qc}")
                nc.vector.tensor_copy(avs[0:64, :], av_a[0:64, :])
                nc.vector.tensor_copy(avs[64:128, :], av_b[64:128, :])

                # denominator chain: reduce the f32 essum with two 1-col
                # fp32 matmuls (rows 0 and 32 of one PSUM bank), reciprocal,
                # broadcast with two concurrent K=1 matmuls, multiply into
                # yt (the col-tiled AV already has the pair's two heads
                # stacked [128,512] -- no partition shifts anywhere).
                def norm_pair(p=p, qc=qc, avs=avs, essum=essum):
                    dps = ps.tile([33, 512], F32, tag="ps", name=f"d_{p}_{qc}")
                    for h in range(2):
                        nc.tensor.matmul(dps[32 * h:32 * h + 1, :], onesf_sb[:, h:h + 1],
                                         essum[:, h * 512:(h + 1) * 512],
                                         start=True, stop=True, tile_position=(0, 32 * h))
                    rcf = rc_pool.tile([33, 512], F32, tag="rcf", name=f"rcf_{p}_{qc}")
                    rc2 = rc_pool.tile([33, 512], BF16, tag="rc", name=f"rc_{p}_{qc}")
                    for h in range(2):
                        nc.vector.reciprocal_approx_fast(rcf[32 * h:32 * h + 1, :],
                                                         dps[32 * h:32 * h + 1, :])
                        nc.vector.tensor_copy(rc2[32 * h:32 * h + 1, :],
                                              rcf[32 * h:32 * h + 1, :])
                    psR = ps.tile([128, 512], F32, tag="ps", name=f"psR_{p}_{qc}")
                    for h in range(2):
                        nc.tensor.matmul(psR[64 * h:64 * (h + 1), :],
                                         ones_sb[32 * h:32 * h + 1, 0:64],
                                         rc2[32 * h:32 * h + 1, :],
                                         start=True, stop=True, tile_position=(32 * h, 64 * h))
                    nc.vector.tensor_mul(yt_sb[:, p * T + qc * 512: p * T + (qc + 1) * 512],
                                         avs[:], psR[:])

                pending.append((g + 2, norm_pair))

            pending += [(g + 2, lambda t=t, qc=qc: emit_proj_group(t, tail=(qc == 3)))
                        for t in range(4 * qc, 4 * qc + 4)]

        for _, fn in pending:
            fn()


_NC_CACHE = None


def _get_nc():
    global _NC_CACHE
    if _NC_CACHE is None:
        _NC_CACHE = build_kernel()
    return _NC_CACHE


def _make_in_maps(x, w_attn, w_proj):
    x = np.asarray(x, dtype=np.float32)
    w_attn = np.asarray(w_attn, dtype=np.float32)
    w_proj = np.asarray(w_proj, dtype=np.float32)
    # trimask: es layout is [key j, query i] within the diagonal 128-block;
    # valid iff j <= i, so keep the inclusive upper triangle, zero the rest.
    tri = np.triu(np.ones((128, 128), dtype=np.float32))
    trimask = np.concatenate([tri, tri], axis=1)
    ones = np.ones((128, 128), dtype=np.float32)
    in_maps = []
    for core in range(NCORES):
        b, g = core // 4, core % 4
        hs = g * HPC
        q_cols = w_attn[:, hs * HD:(hs + HPC) * HD]
        k_cols = w_attn[:, C + hs * HD: C + (hs + HPC) * HD]
        v_cols = w_attn[:, 2 * C + hs * HD: 2 * C + (hs + HPC) * HD]
        wqk = np.concatenate(
            [q_cols[:, 0:128], k_cols[:, 0:128], q_cols[:, 128:256], k_cols[:, 128:256]], axis=1)
        in_maps.append({
            "xT": np.ascontiguousarray(x[b].T).astype(BF16NP),
            "wqk": np.ascontiguousarray(wqk).astype(BF16NP),
            "wv": np.ascontiguousarray(v_cols).astype(BF16NP),
            "wp": np.ascontiguousarray(w_proj[hs * HD:(hs + HPC) * HD, :]).astype(BF16NP),
            "trimask": trimask.astype(BF16NP),
            "ones": ones.astype(BF16NP),
        })
    return in_maps


def run_cores(x, w_attn, w_proj, trace=False):
    nc = _get_nc()
    in_maps = _make_in_maps(x, w_attn, w_proj)
    res = run_bass_kernel_spmd(nc, in_maps, core_ids=list(range(NCORES)), trace=trace)
    out = np.zeros((B, T, C), dtype=np.float32)
    for core in range(NCORES):
        out[core // 4] += np.asarray(res.results[core]["out"], dtype=np.float32)
    return out, res


def kernel(x, w_attn, w_proj):
    out, _ = run_cores(x, w_attn, w_proj, trace=False)
    return out


# revision 22
# speedup vs baseline: 1.1411x; 1.1411x over previous
"""Causal self-attention on 8 Trainium2 NeuronCores.

Sharding: core = (batch b in {0,1}) x (head-group g in {0..3}), 4 heads per
core. Each core computes qkv for its heads from x[b], runs causal attention,
and multiplies by its 256 rows of w_proj, producing a partial [T, C] output
in bf16. Host sums the 4 partials per batch in f32.

Layout: everything is computed "transposed" so no on-chip transposes are
needed. The host feeds x[b].T in bf16; q^T/k^T come out of the qkv matmul
with head-dim on partitions (exactly the S^T = K Q^T operand layout); softmax
is done on S^T (keys on partitions, queries on free); the A@V output Y^T is
exactly the lhsT layout the final projection needs.

v4 (vs the 165us v3):
- AV is col-tiled: head 2p -> array cols 0-63, head 2p+1 -> cols 64-127, the
  two streams run concurrently, halving AV wall time AND landing Y^T with
  the pair's two heads stacked [128, 512] -- which kills the ytr/sums
  partition-shift DMAs outright.
- the softmax denominator (previously a 65th ones-column on V, which blocked
  col-tiling) is now: gpsimd accumulates es into an f32 essum per (pair,qc)
  off the critical path; one tiny PE matmul (ones^T @ essum) reduces it, two
  concurrent K=1 matmuls broadcast the reciprocal to 128 partitions.
- the causal mask is a post-exp DVE multiply with a 0/1 triangle instead of
  PE matmuls accumulating -inf blocks pre-exp.
- exp (ACT) runs one kt ahead of the AV that consumes it, so the ~68us of
  total exp time stays off the PE's critical path.
- no heaters: the drip-feed of B/proj/norm work keeps the PE HAM-warm
  through the drain.
"""

import numpy as np
import ml_dtypes

import concourse.bass as bass
import concourse.bacc as bacc
import concourse.tile as tile
from concourse import mybir
from concourse.bass_utils import run_bass_kernel_spmd

F32 = mybir.dt.float32
BF16 = mybir.dt.bfloat16
EXP = mybir.ActivationFunctionType.Exp
COPY = mybir.ActivationFunctionType.Copy
BF16NP = ml_dtypes.bfloat16

B, T, C, H, HD = 2, 2048, 1024, 16, 64
NCORES = 8
HPC = 4      # heads per core
NPAIR = 2    # head pairs per core
NCT = C // 128   # 8 c-tiles
NTT = T // 128   # 16 t-tiles
NQC = T // 512   # 4 query chunks
SCALE = 1.0 / np.sqrt(HD)


def build_kernel():
    nc = bacc.Bacc("TRN2", target_bir_lowering=False, debug=False, num_devices=NCORES)

    xT = nc.dram_tensor("xT", [C, T], BF16, kind="ExternalInput")
    wqk = nc.dram_tensor("wqk", [C, 512], BF16, kind="ExternalInput")
    wv = nc.dram_tensor("wv", [C, 256], BF16, kind="ExternalInput")
    wp = nc.dram_tensor("wp", [256, C], BF16, kind="ExternalInput")
    trimask = nc.dram_tensor("trimask", [128, 256], BF16, kind="ExternalInput")
    ones = nc.dram_tensor("ones", [128, 128], BF16, kind="ExternalInput")
    sel = nc.dram_tensor("sel", [2, 128], BF16, kind="ExternalInput")
    out = nc.dram_tensor("out", [T, C], BF16, kind="ExternalOutput")
    dbg_rcf = nc.dram_tensor("dbg_rcf", [2, 512], BF16, kind="ExternalOutput")

    with tile.TileContext(nc) as tc:
        _body(tc, xT, wqk, wv, wp, trimask, ones, sel, out, dbg_rcf)

    nc.compile()
    return nc


def _body(tc, xT, wqk, wv, wp, trimask, ones, sel, out, dbg_rcf=None):
    nc = tc.nc
    from contextlib import ExitStack

    with ExitStack() as ctx:
        sb = lambda name: ctx.enter_context(tc.tile_pool(name=name, bufs=1))
        qkT_sb = sb("qkT").tile([128, 4 * T], BF16)       # bands q0,k0,q1,k1
        v_sb = sb("v").tile([128, NTT * 256], BF16)       # per k-tile: 4 heads x 64
        yt_sb = sb("yt").tile([128, NPAIR * T], BF16)     # pair p: rows 0-63 head 2p, 64-127 head 2p+1
        wp_sb = sb("wp").tile([128, 2 * C], BF16)
        trimask_sb = sb("trimask").tile([128, 256], BF16)
        ones_sb = sb("ones").tile([128, 128], BF16)
        sel_sb = sb("sel").tile([2, 128], BF16)

        es_pool = ctx.enter_context(tc.tile_pool(name="es", bufs=3))
        essum_pool = ctx.enter_context(tc.tile_pool(name="essum", bufs=2))
        rc_pool = ctx.enter_context(tc.tile_pool(name="rc", bufs=2))
        ost_pool = ctx.enter_context(tc.tile_pool(name="ost", bufs=3))

        # all-ones f32 column for the fp32 denominator matmuls
        onesf_sb = sb("onesf").tile([128, 2], F32)
        nc.vector.memset(onesf_sb[:], 1.0)

        # PSUM (8 banks): psS 2 slots x [128,1024] = 4, av 2 x [128,512] = 2,
        # misc (B psv / proj pso / d / psR rotate) 2 x [128,512] = 2.
        ps = ctx.enter_context(tc.tile_pool(name="ps", bufs=2, space="PSUM"))
        av_pool = ctx.enter_context(tc.tile_pool(name="av", bufs=2, space="PSUM"))
        psS_pool = ctx.enter_context(tc.tile_pool(name="psS", bufs=2, space="PSUM"))

        xw_pool = ctx.enter_context(tc.tile_pool(name="xw", bufs=1))
        xT_sb = xw_pool.tile([128, NCT * T], BF16, name="xT_sb")
        wqk_sb = xw_pool.tile([128, NCT * 512], BF16, name="wqk_sb")
        wv_sb = xw_pool.tile([128, NCT * 256], BF16, name="wv_sb")

        # input DMAs: few big instructions (each costs ~600ns issue time)
        def load_w(w, w_sb, cols, c0, ncx):
            dst = w_sb[:, c0 * cols:(c0 + ncx) * cols].rearrange("p (c w) -> p c w", c=ncx)
            src = w[c0 * 128:(c0 + ncx) * 128, :].rearrange("(c p) w -> p c w", c=ncx, p=128)
            nc.gpsimd.dma_start(dst, src)

        # x even c-tiles on sync, odd on scalar (idle until the first exp);
        # weights on gpsimd -- three DGE pipelines ramp in parallel so stage
        # A's c-loop never starves. c0 is split in halves so the first
        # stage-A matmuls can start as early as possible.
        def load_x(c0, eng, tlo=0, thi=T):
            eng.dma_start(xT_sb[:, c0 * T + tlo:c0 * T + thi],
                          xT[c0 * 128:(c0 + 1) * 128, tlo:thi])

        load_w(wqk, wqk_sb, 512, 0, 1)
        load_x(0, nc.sync, 0, 1024)
        load_x(0, nc.scalar, 1024, 2048)
        load_w(wv, wv_sb, 256, 0, 1)
        load_x(1, nc.sync)
        load_w(wqk, wqk_sb, 512, 1, 3)
        load_w(wv, wv_sb, 256, 1, 3)
        load_x(2, nc.scalar)
        load_x(3, nc.sync)
        load_w(wqk, wqk_sb, 512, 4, 4)
        load_w(wv, wv_sb, 256, 4, 4)
        load_x(4, nc.scalar)
        load_x(5, nc.sync)
        load_x(6, nc.scalar)
        load_x(7, nc.sync)
        nc.gpsimd.dma_start(
            wp_sb[:].rearrange("p (c w) -> p c w", c=2),
            wp.ap().rearrange("(c p) w -> p c w", c=2, p=128))
        nc.gpsimd.dma_start(trimask_sb[:], trimask[:])
        nc.gpsimd.dma_start(ones_sb[:], ones[:])
        nc.gpsimd.dma_start(sel_sb[:], sel[:])

        # ---- stage A: qkT bands [128, T] = w_band^T @ xT ----
        # c-outer over two-band passes: all 8 PSUM banks hold the 8
        # accumulating t-chunks, each c-tile of xT feeds 8 matmuls the
        # moment its DMA lands, and each lhsT load feeds 4 matmuls.
        for pass_ in range(2):
            bands = (2 * pass_, 2 * pass_ + 1)
            accs = {}
            for b in bands:
                acc01 = psS_pool.tile([128, 1024], F32, tag="psS", name=f"accA_{b}_01")
                acc2 = av_pool.tile([128, 512], F32, tag="av", name=f"accA_{b}_2")
                acc3 = ps.tile([128, 512], F32, tag="ps", name=f"accA_{b}_3")
                accs[b] = [acc01, acc2, acc3]
            for c in range(NCT):
                for b in bands:
                    lhs = wqk_sb[:, c * 512 + b * 128: c * 512 + (b + 1) * 128]
                    acc01, acc2, acc3 = accs[b]
                    dsts = [acc01[:, 0:512], acc01[:, 512:1024], acc2[:], acc3[:]]
                    for t4 in range(4):
                        nc.tensor.matmul(
                            dsts[t4], lhs,
                            xT_sb[:, c * T + t4 * 512: c * T + (t4 + 1) * 512],
                            start=(c == 0), stop=(c == NCT - 1))
            for b in bands:
                acc01, acc2, acc3 = accs[b]
                nc.vector.tensor_copy(qkT_sb[:, b * T: b * T + 1024], acc01[:])
                nc.vector.tensor_copy(qkT_sb[:, b * T + 1024: b * T + 1536], acc2[:])
                nc.vector.tensor_copy(qkT_sb[:, b * T + 1536: b * T + 2048], acc3[:])

        # ---- stage B: v natural [t, j]; tiles 0-3 now, the rest
        # drip-fed into stage C as PE filler (keeps HAM at full clock) --
        B_psv = {}

        def emit_B_half(t, half):
            # halved so one drip-pop (~430ns of PE) fits the per-kt slack
            # under the exp cadence without stalling the S pipeline
            if half == 0:
                B_psv[t] = ps.tile([128, 512], F32, tag="ps", name=f"psv_{t}")
            psv = B_psv[t]
            for c in range(4 * half, 4 * half + 4):
                lhs = xT_sb[:, c * T + t * 128: c * T + (t + 1) * 128]
                nc.tensor.matmul(psv[:, 0:256], lhs, wv_sb[:, c * 256:(c + 1) * 256],
                                 start=(c == 0), stop=(c == NCT - 1))
            if half == 1:
                nc.vector.tensor_copy(v_sb[:, t * 256:(t + 1) * 256], psv[:, 0:256])
                del B_psv[t]

        def emit_B(t):
            emit_B_half(t, 0)
            emit_B_half(t, 1)

        for t in range(4):
            emit_B(t)

        # ---- stage C: attention; stage D: projection. All cross-chunk
        # serial work (denominator chain, projection) is drip-fed into the
        # NEXT chunk's kt loop so the in-order PE stream never stalls >3.4us
        # (a fully-idle HAM window would halve the PE clock for the rest of
        # the attention phase).
        def emit_proj_group(t, tail=False):
            ost = ost_pool.tile([128, 1024], BF16, tag="ost", name=f"ost_{t}")
            for n in range(2):
                pso = ps.tile([128, 512], F32, tag="ps", name=f"pso_{t}_{n}")
                for p in range(NPAIR):
                    lhsT = yt_sb[:, p * T + t * 128: p * T + (t + 1) * 128]
                    rhs = wp_sb[:, p * C + n * 512: p * C + (n + 1) * 512]
                    nc.tensor.matmul(pso[:], lhsT, rhs, start=(p == 0), stop=(p == NPAIR - 1))
                if tail and n == 0:
                    # post-attention drain: ACT is idle, split the PSUM
                    # evictions across scalar+vector
                    nc.scalar.activation(ost[:, 0:512], pso[:], COPY, scale=1.0)
                else:
                    nc.vector.tensor_copy(ost[:, n * 512:(n + 1) * 512], pso[:])
            nc.sync.dma_start(out[t * 128:(t + 1) * 128, :], ost[:])

        # Drip-feed scheduling on a global step counter g (one step per kt
        # iteration across the whole attention phase). Each pending item
        # carries a threshold step; the pop scans for the first eligible
        # item so late-deadline items don't head-of-line-block early ones.
        # B(t) is deadlined to just before the AV that reads v[t].
        OFF_P0 = {0: 0, 1: 8, 2: 24, 3: 48}
        pending = [(max(0, OFF_P0[t // 4] + t - 8 + half), lambda t=t, half=half: emit_B_half(t, half))
                   for t in range(4, NTT) for half in range(2)]
        g = 0

        for qc in (0, 1, 2, 3):
            nkt = 4 * qc + 4
            for p in range(NPAIR):
                qb, kb = 2 * p, 2 * p + 1
                # col-tiled AV: each head's accumulation group must own its
                # bank (two concurrent groups can't share a zero region),
                # h0 lands on partitions 0-63 of bank a, h1 on 64-127 of b.
                av_a = av_pool.tile([128, 512], F32, tag="av", name=f"ava_{p}_{qc}")
                av_b = av_pool.tile([128, 512], F32, tag="av", name=f"avb_{p}_{qc}")
                essum = essum_pool.tile([128, 1024], F32, tag="essum", name=f"essum_{p}_{qc}")

                def emit_S(kt, p=p, qb=qb, kb=kb, qc=qc):
                    psb = psS_pool.tile([128, 1024], F32, tag="psS", name=f"psS_{p}_{qc}_{kt}")
                    d = kt - 4 * qc
                    slo = max(d, 0) * 128
                    for h in range(2):
                        base = 64 * h
                        lhsT = qkT_sb[base:base + 64, kb * T + kt * 128: kb * T + (kt + 1) * 128]
                        rhs = qkT_sb[base:base + 64, qb * T + qc * 512 + slo: qb * T + (qc + 1) * 512]
                        nc.tensor.matmul(psb[:, h * 512 + slo:(h + 1) * 512], lhsT, rhs,
                                         start=True, stop=True, tile_position=(base, 0))
                    return psb

                def emit_exp(kt, psb, p=p, qc=qc, essum=essum):
                    # exp of S^T (ACT), then 0/1-triangle mask on the
                    # diagonal block (DVE), then accumulate into the f32
                    # denominator tile (gpsimd, far off the critical path).
                    d = kt - 4 * qc
                    lo = max(d, 0) * 128
                    es = es_pool.tile([128, 1024], BF16, tag="es", name=f"es_{p}_{qc}_{kt}")
                    es2 = es[:].rearrange("p (h q) -> p h q", h=2, q=512)
                    psb2 = psb[:].rearrange("p (h q) -> p h q", h=2, q=512)
                    nc.scalar.activation(es2[:, :, lo:], psb2[:, :, lo:], EXP, scale=SCALE)
                    if d >= 0:
                        tm = trimask_sb[:].rearrange("p (h q) -> p h q", h=2, q=128)
                        nc.vector.tensor_mul(es2[:, :, lo:lo + 128], es2[:, :, lo:lo + 128], tm)
                    esum2 = essum[:].rearrange("p (h q) -> p h q", h=2, q=512)
                    if kt == 0:
                        nc.gpsimd.tensor_copy(esum2[:, :, lo:], es2[:, :, lo:])
                    else:
                        nc.gpsimd.tensor_tensor(esum2[:, :, lo:], esum2[:, :, lo:], es2[:, :, lo:],
                                                mybir.AluOpType.add)
                    return es

                def emit_AV(kt, es, p=p, qc=qc, av_a=av_a, av_b=av_b, nkt=nkt):
                    d = kt - 4 * qc
                    lo = max(d, 0) * 128
                    for h, av in ((0, av_a), (1, av_b)):
                        hh = 2 * p + h
                        lhsT_v = v_sb[:, kt * 256 + hh * 64: kt * 256 + (hh + 1) * 64]
                        nc.tensor.matmul(av[64 * h:64 * (h + 1), lo:], lhsT_v,
                                         es[:, h * 512 + lo:(h + 1) * 512],
                                         start=(kt == 0), stop=(kt == nkt - 1),
                                         tile_position=(0, 64 * h))

                # software pipeline: S runs 2 kts ahead, exp 1 kt ahead of
                # the AV that consumes it, so ACT (the exp-rate floor of the
                # whole attention phase) never blocks the PE.
                pipe = [emit_S(0)]
                if nkt > 1:
                    pipe.append(emit_S(1))
                es_cur = emit_exp(0, pipe[0])
                for kt in range(nkt):
                    pipe.pop(0)
                    if kt + 2 < nkt:
                        pipe.append(emit_S(kt + 2))
                    es_next = emit_exp(kt + 1, pipe[0]) if kt + 1 < nkt else None
                    emit_AV(kt, es_cur)
                    es_cur = es_next
                    # pop at most ONE filler per kt: a 2-item burst delays
                    # S(kt+2) past the exp cadence and stalls the pipeline
                    i = 0
                    while i < len(pending):
                        if pending[i][0] <= g:
                            pending.pop(i)[1]()
                            break
                        i += 1
                    g += 1

                # gather Y^T to SBUF inline so the pair's av banks free
                # before the next pair's first AV (the av ring is 2 deep)
                avs = rc_pool.tile([128, 512], F32, tag="avs", name=f"avs_{p}_{qc}")
                nc.vector.tensor_copy(avs[0:64, :], av_a[0:64, :])
                nc.vector.tensor_copy(avs[64:128, :], av_b[64:128, :])

                # denominator chain: reduce the f32 essum with two 1-col
                # fp32 matmuls (both at partition 0, separate banks),
                # reciprocal, gather to [2,512] (row 1 via a tiny SBUF-SBUF
                # partition-shift DMA), broadcast with the K=2 sel matmul,
                # multiply into yt (the col-tiled AV already stacked the
                # pair's two heads [128,512]).
                def norm_pair(p=p, qc=qc, avs=avs, essum=essum):
                    dps_a = ps.tile([1, 512], F32, tag="ps", name=f"da_{p}_{qc}")
                    dps_b = ps.tile([1, 512], F32, tag="ps", name=f"db_{p}_{qc}")
                    for h, dps in ((0, dps_a), (1, dps_b)):
                        nc.tensor.matmul(dps[:], onesf_sb[:, h:h + 1],
                                         essum[:, h * 512:(h + 1) * 512],
                                         start=True, stop=True)
                    rcf = rc_pool.tile([1, 1024], F32, tag="rcf", name=f"rcf_{p}_{qc}")
                    rcb = rc_pool.tile([1, 512], BF16, tag="rcb", name=f"rcb_{p}_{qc}")
                    rc2 = rc_pool.tile([2, 512], BF16, tag="rc", name=f"rc_{p}_{qc}")
                    nc.vector.reciprocal_approx_fast(rcf[:, 0:512], dps_a[:])
                    nc.vector.reciprocal_approx_fast(rcf[:, 512:1024], dps_b[:])
                    nc.vector.tensor_copy(rc2[0:1, :], rcf[:, 0:512])
                    nc.vector.tensor_copy(rcb[:], rcf[:, 512:1024])
                    nc.gpsimd.dma_start(rc2[1:2, :], rcb[:])
                    psR = ps.tile([128, 512], F32, tag="ps", name=f"psR_{p}_{qc}")
                    nc.tensor.matmul(psR[:], sel_sb[:], rc2[:], start=True, stop=True)
                    nc.vector.tensor_mul(yt_sb[:, p * T + qc * 512: p * T + (qc + 1) * 512],
                                         avs[:], psR[:])
                    if dbg_rcf is not None and p == 0 and qc == 0:
                        nc.sync.dma_start(dbg_rcf[:], rc2[:])

                pending.append((g + 2, norm_pair))

            pending += [(g + 2, lambda t=t, qc=qc: emit_proj_group(t, tail=(qc == 3)))
                        for t in range(4 * qc, 4 * qc + 4)]

        for _, fn in pending:
            fn()


_NC_CACHE = None


def _get_nc():
    global _NC_CACHE
    if _NC_CACHE is None:
        _NC_CACHE = build_kernel()
    return _NC_CACHE


def _make_in_maps(x, w_attn, w_proj):
    x = np.asarray(x, dtype=np.float32)
    w_attn = np.asarray(w_attn, dtype=np.float32)
    w_proj = np.asarray(w_proj, dtype=np.float32)
    # trimask: es layout is [key j, query i] within the diagonal 128-block;
    # valid iff j <= i, so keep the inclusive upper triangle, zero the rest.
    tri = np.triu(np.ones((128, 128), dtype=np.float32))
    trimask = np.concatenate([tri, tri], axis=1)
    ones = np.ones((128, 128), dtype=np.float32)
    # sel broadcasts the per-head reciprocal row to that head's 64 partitions
    sel = np.zeros((2, 128), dtype=np.float32)
    sel[0, 0:64] = 1.0
    sel[1, 64:128] = 1.0
    in_maps = []
    for core in range(NCORES):
        b, g = core // 4, core % 4
        hs = g * HPC
        q_cols = w_attn[:, hs * HD:(hs + HPC) * HD]
        k_cols = w_attn[:, C + hs * HD: C + (hs + HPC) * HD]
        v_cols = w_attn[:, 2 * C + hs * HD: 2 * C + (hs + HPC) * HD]
        wqk = np.concatenate(
            [q_cols[:, 0:128], k_cols[:, 0:128], q_cols[:, 128:256], k_cols[:, 128:256]], axis=1)
        in_maps.append({
            "xT": np.ascontiguousarray(x[b].T).astype(BF16NP),
            "wqk": np.ascontiguousarray(wqk).astype(BF16NP),
            "wv": np.ascontiguousarray(v_cols).astype(BF16NP),
            "wp": np.ascontiguousarray(w_proj[hs * HD:(hs + HPC) * HD, :]).astype(BF16NP),
            "trimask": trimask.astype(BF16NP),
            "ones": ones.astype(BF16NP),
            "sel": sel.astype(BF16NP),
        })
    return in_maps


def run_cores(x, w_attn, w_proj, trace=False):
    nc = _get_nc()
    in_maps = _make_in_maps(x, w_attn, w_proj)
    res = run_bass_kernel_spmd(nc, in_maps, core_ids=list(range(NCORES)), trace=trace)
    out = np.zeros((B, T, C), dtype=np.float32)
    for core in range(NCORES):
        out[core // 4] += np.asarray(res.results[core]["out"], dtype=np.float32)
    return out, res


def kernel(x, w_attn, w_proj):
    out, _ = run_cores(x, w_attn, w_proj, trace=False)
    return out
